# revision 2
# baseline (speedup 1.0000x reference)
"""Trainium2 Bass kernel for CubeFaceNN.

Computes, for x of shape [8, 1, 128, 128, 128] (f32):
    out[b, i, p] = relu(x[b, 0, p] - x[b, 0, p + OFF[i]])   (zero padded)
with OFF = [(0,-1,-1), (-1,0,-1), (1,-1,-1), (-1,1,-1), (-1,-1,0), (-1,-1,1)]
(derived from the reference's adj % 3 - 1 indexing).

Sharding: pure data parallel — batch b -> NeuronCore b (8 cores).

Per-core layout: depth d on the 128 SBUF partitions, (h, w) in the free
dims, processed in h-chunks. Compute engines cannot address SBUF at a
partition offset of 1, so a partition-shifted copy xp[d] = x[d+1] is built
with a SBUF->SBUF DMA per chunk. Channels with od = -1 are computed in the
substituted form out[i, d'+1] = relu(xp[d'] - x[d', h+oh, w+ow]) so that a
single shifted copy serves all five d-shifting channels.
"""

import numpy as np

import concourse.bacc as bacc
import concourse.mybir as mybir
import concourse.tile as tile
from concourse.bass_utils import run_bass_kernel_spmd

D = H = W = 128
N_CORES = 8
HC = 32  # h-chunk size
F32 = mybir.dt.float32

# (od, oh, ow) per output channel
OFFSETS = [(0, -1, -1), (-1, 0, -1), (1, -1, -1), (-1, 1, -1), (-1, -1, 0), (-1, -1, 1)]

_NC_CACHE = {}


def build_nc(debug=False):
    nc = bacc.Bacc("TRN2", target_bir_lowering=False, debug=debug)
    x = nc.dram_tensor("x", [D, H, W], F32, kind="ExternalInput")
    out = nc.dram_tensor("out", [6, D, H, W], F32, kind="ExternalOutput")

    sub = mybir.AluOpType.subtract
    relu = mybir.ActivationFunctionType.Relu

    with tile.TileContext(nc) as tc:
        with (
            tc.tile_pool(name="xt", bufs=2) as xt_pool,
            tc.tile_pool(name="xp", bufs=2) as xp_pool,
            tc.tile_pool(name="och", bufs=6) as och_pool,
            tc.tile_pool(name="plane", bufs=2) as plane_pool,
        ):
            # d-boundary planes: out[i, 0] = relu(x[0]) for od=-1 channels,
            # out[2, 127] = relu(x[127]). Loaded with h on partitions so the
            # relu runs on 128 partitions (128 cycles, not 16K).
            p0 = plane_pool.tile([H, W], F32)
            nc.sync.dma_start(out=p0[:], in_=x[0])
            nc.vector.tensor_scalar_max(p0[:], p0[:], 0.0)
            for i, (od, _, _) in enumerate(OFFSETS):
                if od == -1:
                    nc.sync.dma_start(out=out[i, 0], in_=p0[:])
            p1 = plane_pool.tile([H, W], F32)
            nc.sync.dma_start(out=p1[:], in_=x[D - 1])
            nc.vector.tensor_scalar_max(p1[:], p1[:], 0.0)
            nc.sync.dma_start(out=out[2, D - 1], in_=p1[:])

            for c in range(H // HC):
                h0 = c * HC
                lo = max(0, h0 - 1)  # first loaded row (halo)
                hi = min(H, h0 + HC + 1)  # one past last loaded row
                nr = hi - lo

                xt = xt_pool.tile([D, nr, W], F32)
                nc.sync.dma_start(out=xt[:], in_=x[:, lo:hi, :])
                # xp[d] = x[d+1] on partitions 0..126 (SBUF->SBUF shift)
                xp = xp_pool.tile([D, nr, W], F32)
                nc.sync.dma_start(out=xp[0 : D - 1], in_=xt[1:D])

                for i, (od, oh, ow) in enumerate(OFFSETS):
                    # aligned operand A and its d-shift in the stored frame
                    if od == -1:
                        A, S, dc = xp, xt, D - 1  # substituted frame
                    elif od == 1:
                        A, S, dc = xt, xp, D - 1
                    else:
                        A, S, dc = xt, xt, D

                    # valid output h rows within this chunk (h + oh in range)
                    hs = max(h0, -oh)
                    he = min(h0 + HC, H - max(0, oh))
                    ws = max(0, -ow)
                    we = W - max(0, ow)

                    och = och_pool.tile([D, HC, W], F32)
                    nc.vector.tensor_tensor(
                        out=och[0:dc, hs - h0 : he - h0, ws:we],
                        in0=A[0:dc, hs - lo : he - lo, ws:we],
                        in1=S[0:dc, hs + oh - lo : he + oh - lo, ws + ow : we + ow],
                        op=sub,
                    )
                    nc.scalar.activation(
                        och[0:dc, hs - h0 : he - h0, ws:we],
                        och[0:dc, hs - h0 : he - h0, ws:we],
                        relu,
                    )
                    # boundary strips: shifted source is zero there -> relu(A)
                    if oh == -1 and h0 == 0:
                        nc.vector.tensor_scalar_max(
                            och[0:dc, 0:1, :], A[0:dc, 0 - lo : 1 - lo, :], 0.0
                        )
                    if oh == 1 and h0 + HC == H:
                        nc.vector.tensor_scalar_max(
                            och[0:dc, HC - 1 : HC, :],
                            A[0:dc, H - 1 - lo : H - lo, :],
                            0.0,
                        )
                    if ow != 0:
                        wb = 0 if ow == -1 else W - 1
                        nc.vector.tensor_scalar_max(
                            och[0:dc, hs - h0 : he - h0, wb : wb + 1],
                            A[0:dc, hs - lo : he - lo, wb : wb + 1],
                            0.0,
                        )

                    od_orig = OFFSETS[i][0]
                    if od_orig == -1:
                        dst = out[i, 1:D, h0 : h0 + HC, :]
                    elif od_orig == 1:
                        dst = out[i, 0 : D - 1, h0 : h0 + HC, :]
                    else:
                        dst = out[i, :, h0 : h0 + HC, :]
                    nc.sync.dma_start(out=dst, in_=och[0:dc])

    nc.compile()
    return nc


def _get_nc():
    if "nc" not in _NC_CACHE:
        _NC_CACHE["nc"] = build_nc()
    return _NC_CACHE["nc"]


def kernel(x: np.ndarray) -> np.ndarray:
    assert x.shape == (N_CORES, 1, D, H, W), x.shape
    nc = _get_nc()
    in_maps = [{"x": np.ascontiguousarray(x[b, 0], dtype=np.float32)} for b in range(N_CORES)]
    res = run_bass_kernel_spmd(nc, in_maps, core_ids=list(range(N_CORES)))
    return np.stack([r["out"] for r in res.results], axis=0)


# revision 3
# speedup vs baseline: 2.8178x; 2.8178x over previous
"""Trainium2 Bass kernel for CubeFaceNN.

Computes, for x of shape [8, 1, 128, 128, 128] (f32):
    out[b, i, p] = relu(x[b, 0, p] - x[b, 0, p + OFF[i]])   (zero padded)
with OFF = [(0,-1,-1), (-1,0,-1), (1,-1,-1), (-1,1,-1), (-1,-1,0), (-1,-1,1)]
(derived from the reference's adj % 3 - 1 indexing).

Sharding: pure data parallel — batch b -> NeuronCore b (8 cores).

Per-core layout: depth d on the 128 SBUF partitions, (h, w) in the free
dims, processed in h-chunks. Compute engines cannot address SBUF at a
partition offset of 1, so a partition-shifted copy xp[d] = x[d+1] is built
with a SBUF->SBUF DMA per chunk. Channels with od = -1 are computed in the
substituted form out[i, d'+1] = relu(xp[d'] - x[d', h+oh, w+ow]) so that a
single shifted copy serves all five d-shifting channels.
"""

import numpy as np

import concourse.bacc as bacc
import concourse.mybir as mybir
import concourse.tile as tile
from concourse.bass_utils import run_bass_kernel_spmd

D = H = W = 128
N_CORES = 8
HC = 32  # h-chunk size
F32 = mybir.dt.float32

# (od, oh, ow) per output channel
OFFSETS = [(0, -1, -1), (-1, 0, -1), (1, -1, -1), (-1, 1, -1), (-1, -1, 0), (-1, -1, 1)]

_NC_CACHE = {}


def build_nc(debug=False):
    nc = bacc.Bacc("TRN2", target_bir_lowering=False, debug=debug)
    x = nc.dram_tensor("x", [D, H, W], F32, kind="ExternalInput")
    out = nc.dram_tensor("out", [6, D, H, W], F32, kind="ExternalOutput")

    sub = mybir.AluOpType.subtract
    relu = mybir.ActivationFunctionType.Relu

    with tile.TileContext(nc) as tc:
        with (
            tc.tile_pool(name="xt", bufs=2) as xt_pool,
            tc.tile_pool(name="xp", bufs=2) as xp_pool,
            tc.tile_pool(name="och", bufs=6) as och_pool,
            tc.tile_pool(name="plane", bufs=2) as plane_pool,
        ):
            # d-boundary planes: out[i, 0] = relu(x[0]) for od=-1 channels,
            # out[2, 127] = relu(x[127]). Loaded with h on partitions so the
            # relu runs on 128 partitions (128 cycles, not 16K).
            p0 = plane_pool.tile([H, W], F32)
            nc.sync.dma_start(out=p0[:], in_=x[0])
            nc.vector.tensor_scalar_max(p0[:], p0[:], 0.0)
            for i, (od, _, _) in enumerate(OFFSETS):
                if od == -1:
                    nc.sync.dma_start(out=out[i, 0], in_=p0[:])
            p1 = plane_pool.tile([H, W], F32)
            nc.sync.dma_start(out=p1[:], in_=x[D - 1])
            nc.vector.tensor_scalar_max(p1[:], p1[:], 0.0)
            nc.sync.dma_start(out=out[2, D - 1], in_=p1[:])

            for c in range(H // HC):
                h0 = c * HC
                lo = max(0, h0 - 1)  # first loaded row (halo)
                hi = min(H, h0 + HC + 1)  # one past last loaded row
                nr = hi - lo

                xt = xt_pool.tile([D, nr, W], F32)
                nc.sync.dma_start(out=xt[:], in_=x[:, lo:hi, :])
                # xp[d] = x[d+1] on partitions 0..126 (SBUF->SBUF shift)
                xp = xp_pool.tile([D, nr, W], F32)
                nc.sync.dma_start(out=xp[0 : D - 1], in_=xt[1:D])

                for i, (od, oh, ow) in enumerate(OFFSETS):
                    # aligned operand A and its d-shift in the stored frame
                    if od == -1:
                        A, S, dc = xp, xt, D - 1  # substituted frame
                    elif od == 1:
                        A, S, dc = xt, xp, D - 1
                    else:
                        A, S, dc = xt, xt, D

                    # valid output h rows within this chunk (h + oh in range)
                    hs = max(h0, -oh)
                    he = min(h0 + HC, H - max(0, oh))
                    ws = max(0, -ow)
                    we = W - max(0, ow)

                    och = och_pool.tile([D, HC, W], F32)
                    nc.vector.tensor_tensor(
                        out=och[0:dc, hs - h0 : he - h0, ws:we],
                        in0=A[0:dc, hs - lo : he - lo, ws:we],
                        in1=S[0:dc, hs + oh - lo : he + oh - lo, ws + ow : we + ow],
                        op=sub,
                    )
                    nc.scalar.activation(
                        och[0:dc, hs - h0 : he - h0, ws:we],
                        och[0:dc, hs - h0 : he - h0, ws:we],
                        relu,
                    )
                    # boundary strips: shifted source is zero there -> relu(A)
                    if oh == -1 and h0 == 0:
                        nc.vector.tensor_scalar_max(
                            och[0:dc, 0:1, :], A[0:dc, 0 - lo : 1 - lo, :], 0.0
                        )
                    if oh == 1 and h0 + HC == H:
                        nc.vector.tensor_scalar_max(
                            och[0:dc, HC - 1 : HC, :],
                            A[0:dc, H - 1 - lo : H - lo, :],
                            0.0,
                        )
                    if ow != 0:
                        wb = 0 if ow == -1 else W - 1
                        nc.vector.tensor_scalar_max(
                            och[0:dc, hs - h0 : he - h0, wb : wb + 1],
                            A[0:dc, hs - lo : he - lo, wb : wb + 1],
                            0.0,
                        )

                    od_orig = OFFSETS[i][0]
                    if od_orig == -1:
                        dst = out[i, 1:D, h0 : h0 + HC, :]
                    elif od_orig == 1:
                        dst = out[i, 0 : D - 1, h0 : h0 + HC, :]
                    else:
                        dst = out[i, :, h0 : h0 + HC, :]
                    # SWDGE for the big stores: HWDGE assigns SBUF->HBM
                    # descriptors of this strided AP to a single SDMA engine
                    # (~27 GB/s); the gpsimd CounterMachine swizzles across
                    # all 16.
                    nc.gpsimd.dma_start(out=dst, in_=och[0:dc])

    nc.compile()
    return nc


def _get_nc():
    if "nc" not in _NC_CACHE:
        _NC_CACHE["nc"] = build_nc()
    return _NC_CACHE["nc"]


def kernel(x: np.ndarray) -> np.ndarray:
    assert x.shape == (N_CORES, 1, D, H, W), x.shape
    nc = _get_nc()
    in_maps = [{"x": np.ascontiguousarray(x[b, 0], dtype=np.float32)} for b in range(N_CORES)]
    res = run_bass_kernel_spmd(nc, in_maps, core_ids=list(range(N_CORES)))
    return np.stack([r["out"] for r in res.results], axis=0)


# revision 5
# speedup vs baseline: 2.9659x; 1.0526x over previous
"""Trainium2 Bass kernel for CubeFaceNN.

Computes, for x of shape [8, 1, 128, 128, 128] (f32):
    out[b, i, p] = relu(x[b, 0, p] - x[b, 0, p + OFF[i]])   (zero padded)
with OFF = [(0,-1,-1), (-1,0,-1), (1,-1,-1), (-1,1,-1), (-1,-1,0), (-1,-1,1)]
(derived from the reference's adj % 3 - 1 indexing).

Sharding: pure data parallel — batch b -> NeuronCore b (8 cores).

Per-core layout: depth d on the 128 SBUF partitions, (h, w) in the free
dims, processed in h-chunks. Compute engines cannot address SBUF at a
partition offset of 1, so a partition-shifted copy xp[d] = x[d+1] is built
with a SBUF->SBUF DMA per chunk. Channels with od = -1 are computed in the
substituted form out[i, d'+1] = relu(xp[d'] - x[d', h+oh, w+ow]) so that a
single shifted copy serves all five d-shifting channels.
"""

import numpy as np

import concourse.bacc as bacc
import concourse.mybir as mybir
import concourse.tile as tile
from concourse.bass_utils import run_bass_kernel_spmd

D = H = W = 128
N_CORES = 8
HC = 32  # h-chunk size
F32 = mybir.dt.float32

# (od, oh, ow) per output channel
OFFSETS = [(0, -1, -1), (-1, 0, -1), (1, -1, -1), (-1, 1, -1), (-1, -1, 0), (-1, -1, 1)]

_NC_CACHE = {}


def build_nc(debug=False):
    nc = bacc.Bacc("TRN2", target_bir_lowering=False, debug=debug)
    x = nc.dram_tensor("x", [D, H, W], F32, kind="ExternalInput")
    out = nc.dram_tensor("out", [6, D, H, W], F32, kind="ExternalOutput")

    sub = mybir.AluOpType.subtract
    relu = mybir.ActivationFunctionType.Relu

    with tile.TileContext(nc) as tc:
        with (
            tc.tile_pool(name="xt", bufs=2) as xt_pool,
            tc.tile_pool(name="xp", bufs=2) as xp_pool,
            tc.tile_pool(name="och", bufs=6) as och_pool,
            tc.tile_pool(name="plane", bufs=2) as plane_pool,
        ):
            # d-boundary planes: out[i, 0] = relu(x[0]) for od=-1 channels,
            # out[2, 127] = relu(x[127]). Loaded with h on partitions so the
            # relu runs on 128 partitions (128 cycles, not 16K).
            p0 = plane_pool.tile([H, W], F32)
            nc.sync.dma_start(out=p0[:], in_=x[0])
            nc.vector.tensor_scalar_max(p0[:], p0[:], 0.0)
            for i, (od, _, _) in enumerate(OFFSETS):
                if od == -1:
                    nc.sync.dma_start(out=out[i, 0], in_=p0[:])
            p1 = plane_pool.tile([H, W], F32)
            nc.sync.dma_start(out=p1[:], in_=x[D - 1])
            nc.vector.tensor_scalar_max(p1[:], p1[:], 0.0)
            nc.sync.dma_start(out=out[2, D - 1], in_=p1[:])

            for c in range(H // HC):
                h0 = c * HC
                lo = max(0, h0 - 1)  # first loaded row (halo)
                hi = min(H, h0 + HC + 1)  # one past last loaded row
                nr = hi - lo

                # SWDGE (gpsimd) for all big transfers: the HWDGE dynamic
                # ring drains through a single SDMA engine (~27 GB/s), while
                # the SWDGE CounterMachine swizzles descriptors across all 16.
                xt = xt_pool.tile([D, nr, W], F32)
                nc.gpsimd.dma_start(out=xt[:], in_=x[:, lo:hi, :])
                # xp[d] = x[d+1] on partitions 0..126 (SBUF->SBUF shift)
                xp = xp_pool.tile([D, nr, W], F32)
                nc.gpsimd.dma_start(out=xp[0 : D - 1], in_=xt[1:D])

                for i, (od, oh, ow) in enumerate(OFFSETS):
                    # aligned operand A and its d-shift in the stored frame
                    if od == -1:
                        A, S, dc = xp, xt, D - 1  # substituted frame
                    elif od == 1:
                        A, S, dc = xt, xp, D - 1
                    else:
                        A, S, dc = xt, xt, D

                    # valid output h rows within this chunk (h + oh in range)
                    hs = max(h0, -oh)
                    he = min(h0 + HC, H - max(0, oh))
                    ws = max(0, -ow)
                    we = W - max(0, ow)

                    och = och_pool.tile([D, HC, W], F32)
                    nc.vector.tensor_tensor(
                        out=och[0:dc, hs - h0 : he - h0, ws:we],
                        in0=A[0:dc, hs - lo : he - lo, ws:we],
                        in1=S[0:dc, hs + oh - lo : he + oh - lo, ws + ow : we + ow],
                        op=sub,
                    )
                    nc.scalar.activation(
                        och[0:dc, hs - h0 : he - h0, ws:we],
                        och[0:dc, hs - h0 : he - h0, ws:we],
                        relu,
                    )
                    # boundary strips: shifted source is zero there -> relu(A)
                    if oh == -1 and h0 == 0:
                        nc.vector.tensor_scalar_max(
                            och[0:dc, 0:1, :], A[0:dc, 0 - lo : 1 - lo, :], 0.0
                        )
                    if oh == 1 and h0 + HC == H:
                        nc.vector.tensor_scalar_max(
                            och[0:dc, HC - 1 : HC, :],
                            A[0:dc, H - 1 - lo : H - lo, :],
                            0.0,
                        )
                    if ow != 0:
                        wb = 0 if ow == -1 else W - 1
                        nc.vector.tensor_scalar_max(
                            och[0:dc, hs - h0 : he - h0, wb : wb + 1],
                            A[0:dc, hs - lo : he - lo, wb : wb + 1],
                            0.0,
                        )

                    od_orig = OFFSETS[i][0]
                    if od_orig == -1:
                        dst = out[i, 1:D, h0 : h0 + HC, :]
                    elif od_orig == 1:
                        dst = out[i, 0 : D - 1, h0 : h0 + HC, :]
                    else:
                        # 128-partition stores hit a 4-engine swizzle path;
                        # 127-partition stores spread over all 16 engines, so
                        # peel the last partition into its own tiny store.
                        dst = out[i, 0 : D - 1, h0 : h0 + HC, :]
                        nc.gpsimd.dma_start(
                            out=out[i, D - 1 : D, h0 : h0 + HC, :],
                            in_=och[D - 1 : D],
                        )
                    nc.gpsimd.dma_start(out=dst, in_=och[0 : D - 1])

    nc.compile()
    return nc


def _get_nc():
    if "nc" not in _NC_CACHE:
        _NC_CACHE["nc"] = build_nc()
    return _NC_CACHE["nc"]


def kernel(x: np.ndarray) -> np.ndarray:
    assert x.shape == (N_CORES, 1, D, H, W), x.shape
    nc = _get_nc()
    in_maps = [{"x": np.ascontiguousarray(x[b, 0], dtype=np.float32)} for b in range(N_CORES)]
    res = run_bass_kernel_spmd(nc, in_maps, core_ids=list(range(N_CORES)))
    return np.stack([r["out"] for r in res.results], axis=0)
